# revision 50
# baseline (speedup 1.0000x reference)
"""TRN2 Bass kernel for nn_Baseline_v4_74947179315922 (dense CNN, 22 conv layers).

Network: in_conv(k=32,s=2 on 31x-iterated-wrap-pad) -> 20 hidden convs
(k=32,s=2, ReLU) -> 1x1 out conv.  B=16, grid 32x32, 32 channels.

Strategy (per core, pure data parallelism, B_core=2):
 - The iterated wrap-pad (94x94) is parity-split into four 47x47 planes
   Q[rho,gamma][t,u] = h_pad[2t+rho, 2u+gamma]; the stride-2 32x32-kernel conv
   becomes 256 stride-1 "taps" (a,c) in [0,16)^2 contracting K=128 =
   (4 parities x 32 channels).
 - Border folding: Q[t,u] = P[u] + G[t,u] where P is the (constant) padded
   border row and G is supported on the 16 data rows only.  Each tap then
   streams just the <=16 valid output rows from G (half the dense stream),
   and the border term collapses to 16 "column taps" with row-summed weights
   and a y-broadcast (stride-0) rhs read of P.
 - Each pass runs 4 taps concurrently on the PE array via column tiling
   (tile_position=(0,32j)); PSUM accumulates across all taps; a
   [tile(eye(32),(4,4))] matmul reduces the 4 tap-groups.  The two y-half
   chunks of a tap share one weight slice and alternate PSUM banks, which
   pipelines weight loads and drains (measured ~14ns/MM overhead vs the
   213ns N=512 fp16 stream floor).
 - ReLU+bias applied by ScalarE straight out of PSUM; the next layer's P and
   G are built by a few DVE subtract/copy ops.  The epilogue chain is
   emitted at reduced scheduler priority (tc.high_priority(offset=-200)) so
   the next sample's tap stream slides ahead of it in the engine queues and
   hides the copy->reduce->act->build latency.
"""

import os
import sys

sys.path.insert(0, "/opt/trn_rl_repo")

from contextlib import ExitStack

import numpy as np

import concourse.bass as bass
import concourse.bacc as bacc
import concourse.tile as tile
from concourse import mybir
from concourse.bass_utils import run_bass_kernel_spmd

F32 = mybir.dt.float32
F16 = mybir.dt.float16
RELU = mybir.ActivationFunctionType.Relu
IDENT = mybir.ActivationFunctionType.Identity

N_CORES = 8
B = 16
BC = B // N_CORES          # samples per core
G = 32                     # grid
T47 = 47                   # parity-plane extent
N_HID = 20
GAIN = float(np.sqrt(2.0))   # per-layer activation renorm (ReLU halves variance)

# parity-plane row/col maps: plane rho=0 rows are [31]*16, 1,3..31, [31]*15;
# plane rho=1 rows are [0]*16, 2,4..30, [0]*16  (same for columns)
PMAP = {
    0: [31] * 16 + list(range(1, 32, 2)) + [31] * 15,
    1: [0] * 16 + list(range(2, 31, 2)) + [0] * 16,
}
# bands: (dst_start, dst_len, src_start, src_step)
BANDS = {
    0: [(0, 16, 31, 0), (16, 16, 1, 2), (32, 15, 31, 0)],
    1: [(0, 16, 0, 0), (16, 15, 2, 2), (31, 16, 0, 0)],
}


def _prep_weights(in_w, convs_w):
    """Host-side rearrangement into PE lhsT layouts."""
    n_hid = convs_w.shape[0]
    # hidden: [l, k=(rho*2+gamma)*32+i, p*128 + j*32 + o], tap = 4p+j = a*16+c
    w6 = convs_w.reshape(n_hid, 32, 32, 16, 2, 16, 2)      # l,o,i,a,rho,c,gamma
    wh6 = w6.transpose(0, 4, 6, 2, 3, 5, 1)                # l,rho,gamma,i,a,c,o
    wh = np.ascontiguousarray(wh6.reshape(n_hid, 128, 64 * 128), np.float32)
    # column taps: wc[l, k, c*32+o] = sum_a W  (border-row folding)
    wc = np.ascontiguousarray(
        wh6.sum(axis=4).reshape(n_hid, 128, 16 * 32), np.float32)
    # layer0: [k=(delta*4+rho*2+gamma), p*128 + j*32 + o], a=a0+delta, c=c0+j,
    # a0=(p//4)*8, c0=(p%4)*4
    u = in_w[:, 0].reshape(32, 2, 8, 2, 4, 4, 2)           # o,a0i,delta,rho,c0i,j,gamma
    w0 = u.transpose(2, 3, 6, 1, 4, 5, 0)                  # delta,rho,gamma,a0i,c0i,j,o
    w0 = np.ascontiguousarray(w0.reshape(32, 8 * 128), np.float32)
    return w0, wh, wc


def _prep_q0x(x):
    """Host-built layer-0 input: delta-shifted parity planes of padded x."""
    q0x = np.zeros((32, BC, T47, T47), np.float32)
    for g in range(4):
        rho, gam = g // 2, g % 2
        plane = x[:, 0][:, PMAP[rho]][:, :, PMAP[gam]]     # [BC,47,47]
        for d in range(8):
            q0x[4 * d + g, :, :T47 - d, :] = plane[:, d:, :]
    return q0x


def _build_nc(n_hid=N_HID):
    nc = bacc.Bacc("TRN2", target_bir_lowering=False)

    q0_d = nc.dram_tensor("q0x", [32, BC, T47, T47], F16, kind="ExternalInput")
    w0_d = nc.dram_tensor("w0", [32, 8 * 128], F16, kind="ExternalInput")
    wh_d = nc.dram_tensor("wh", [n_hid, 128, 64 * 128], F16, kind="ExternalInput")
    wc_d = nc.dram_tensor("wc", [n_hid, 128, 16 * 32], F16, kind="ExternalInput")
    # bias rows: 0 = in_b tiled x4; 1..n_hid = convs_b tiled x4; n_hid+1 = out_b
    bia_d = nc.dram_tensor("bia", [n_hid + 2, 128], F32, kind="ExternalInput")
    s_d = nc.dram_tensor("smat", [128, 128], F16, kind="ExternalInput")
    wo_d = nc.dram_tensor("wo", [32, 1], F16, kind="ExternalInput")
    out_d = nc.dram_tensor("out", [BC, 1, G, G], F32, kind="ExternalOutput")

    with tile.TileContext(nc) as tc, ExitStack() as ctx:
        const = ctx.enter_context(tc.tile_pool(name="const", bufs=1))
        gpool = ctx.enter_context(tc.tile_pool(name="gpool", bufs=2))
        ppool = ctx.enter_context(tc.tile_pool(name="ppool", bufs=2))
        wpool = ctx.enter_context(tc.tile_pool(name="wpool", bufs=2))
        wcpool = ctx.enter_context(tc.tile_pool(name="wcpool", bufs=2))
        s1pool = ctx.enter_context(tc.tile_pool(name="s1pool", bufs=2))
        actpool = ctx.enter_context(tc.tile_pool(name="actpool", bufs=2))
        accp = ctx.enter_context(tc.tile_pool(name="accp", bufs=4, space="PSUM"))
        p2p = ctx.enter_context(tc.tile_pool(name="p2p", bufs=3, space="PSUM"))

        q0x = const.tile([32, BC, T47, T47], F16)
        nc.sync.dma_start(q0x[:], q0_d[:])
        w0_t = const.tile([32, 8 * 128], F16)
        nc.sync.dma_start(w0_t[:], w0_d[:])
        s_t = const.tile([128, 128], F16)
        nc.sync.dma_start(s_t[:], s_d[:])
        bia_t = const.tile([128, n_hid + 2], F32)
        nc.sync.dma_start(bia_t[:], bia_d.rearrange("l p -> p l"))
        wo_t = const.tile([32, 1], F16)
        nc.sync.dma_start(wo_t[:], wo_d[:])
        osb = const.tile([1, BC * G * G], F32)

        def chunk_epilogue(l, b, yh, acc, func):
            """reduce 4 tap groups of one chunk, bias+act into act_b."""
            s1 = s1pool.tile([128, 512], F16, tag="s1")
            nc.vector.tensor_copy(s1[:], acc[:])
            p2 = p2p.tile([128, 512], F32, tag="p2")
            nc.tensor.matmul(p2[:], s_t[:, :],
                             s1[:, :], start=True, stop=True)
            nc.scalar.activation(
                act_bs[b][:, yh * 512:(yh + 1) * 512],
                p2[:], func, bias=bia_t[:, l:l + 1])

        def build_pg(l, b, pn, gn):
            """build next layer's P (border row) and G (centered data rows)."""
            act3 = act_bs[b].rearrange("p (r c) -> p r c", r=G)
            # P: padded border row per plane (row 31 for rho=0, row 0 for rho=1)
            for g in range(4):
                rho, gam = g // 2, g % 2
                R = 31 if rho == 0 else 0
                for u0, ul, us, ust in BANDS[gam]:
                    if ust == 0:
                        src = act3[32 * g:32 * (g + 1), R, us:us + 1]
                        src = src.broadcast_to([32, ul])
                    else:
                        src = act3[32 * g:32 * (g + 1), R, us:us + 2 * ul - 1:2]
                    nc.vector.tensor_copy(
                        pn[32 * g:32 * (g + 1), b, u0:u0 + ul], src)
            # G bands: center = data - P; borders = rowborder - corner
            for g in range(4):
                rho, gam = g // 2, g % 2
                nr = 16 if rho == 0 else 15
                rs = 1 if rho == 0 else 2
                C = 31 if gam == 0 else 0
                rows = act3[32 * g:32 * (g + 1), rs:rs + 2 * nr - 1:2, :]
                u0c, wc_, usc, _ = BANDS[gam][1]
                nc.vector.tensor_sub(
                    gn[32 * g:32 * (g + 1), b, 0:nr, u0c:u0c + wc_],
                    rows[:, :, usc:usc + 2 * wc_ - 1:2],
                    pn[32 * g:32 * (g + 1), b, u0c:u0c + wc_]
                    .unsqueeze(1).broadcast_to([32, nr, wc_]))
                for u0, ul, us, ust in (BANDS[gam][0], BANDS[gam][2]):
                    nc.vector.tensor_sub(
                        gn[32 * g:32 * (g + 1), b, 0:nr, u0:u0 + ul],
                        rows[:, :, C:C + 1].broadcast_to([32, nr, ul]),
                        pn[32 * g:32 * (g + 1), b, 0:1]
                        .unsqueeze(1).broadcast_to([32, nr, ul]))

        def taps_l0(accs2, q_ap, b):
            """dense layer-0 taps (K=32, delta folded into K, 8 passes)."""
            for p in range(8):
                for j in range(4):
                    a = (p // 4) * 8
                    c = (p % 4) * 4 + j
                    w_ap = w0_t[:, p * 128 + 32 * j:p * 128 + 32 * (j + 1)]
                    for yh in range(2):
                        rhs = q_ap[0:32, b, yh * 16 + a:yh * 16 + a + 16,
                                   c:c + 32]
                        nc.tensor.matmul(
                            accs2[yh][32 * j:32 * (j + 1), :],
                            w_ap, rhs,
                            start=(p == 0), stop=(p == 7),
                            tile_position=(0, 32 * j), skip_group_check=True)

        def taps_tg(accs2, g_t, p_t, w_t, wc_t, b):
            """hidden-layer taps on G (16 data rows) + P column taps.

            Per-tap bank alternation (yh1 then yh0) pipelines PSUM drains.
            P column taps run FIRST: they only need P (built before G in the
            previous layer's build_pg), so the PE starts this sample's stream
            before its G tile finishes building at layer boundaries.
            """
            # P column taps: y-broadcast rhs, row-summed weights
            for pc in range(4):
                for j in range(4):
                    c = pc * 4 + j
                    wc_ap = wc_t[:, c * 32:(c + 1) * 32]
                    rhs = (p_t[0:128, b, c:c + 32]
                           .unsqueeze(1).broadcast_to([128, 16, 32]))
                    for yh in range(2):
                        nc.tensor.matmul(
                            accs2[yh][32 * j:32 * (j + 1), :],
                            wc_ap, rhs,
                            start=(pc == 0), stop=False,
                            tile_position=(0, 32 * j), skip_group_check=True)
            for p in range(64):
                for j in range(4):
                    tau = 4 * p + j
                    a, c = tau // 16, tau % 16
                    w_ap = w_t[:, p * 128 + 32 * j:p * 128 + 32 * (j + 1)]
                    # chunk yh1: out rows [16,32-a), G rows [a,16)
                    nc.tensor.matmul(
                        accs2[1][32 * j:32 * (j + 1), 0:(16 - a) * 32],
                        w_ap, g_t[0:128, b, a:16, c:c + 32],
                        start=False, stop=(p == 63),
                        tile_position=(0, 32 * j), skip_group_check=True)
                    # chunk yh0: out rows [16-a,16), G rows [0,a)
                    if a >= 1:
                        nc.tensor.matmul(
                            accs2[0][32 * j:32 * (j + 1), (16 - a) * 32:512],
                            w_ap, g_t[0:128, b, 0:a, c:c + 32],
                            start=False, stop=(p == 63),
                            tile_position=(0, 32 * j), skip_group_check=True)

        # ---- all conv layers (layer 0 + hidden); per-b epilogue interleave
        g_cur = p_cur = None
        act_bs = None
        for l in range(n_hid + 1):
            if l == 0:
                w_t = wc_t = None
                func = IDENT
            else:
                w_t = wpool.tile([128, 64 * 128], F16, tag="wh", name=f"wh{l}")
                if l == 1:
                    # first weight load is exposed right after the short
                    # layer 0: split it over the scalar + sync DGE queues
                    nc.scalar.dma_start(w_t[0:64, :], wh_d[l - 1][0:64, :])
                    nc.sync.dma_start(w_t[64:128, :], wh_d[l - 1][64:128, :])
                else:
                    nc.sync.dma_start(w_t[:], wh_d[l - 1])
                wc_t = wcpool.tile([128, 16 * 32], F16, tag="wc", name=f"wc{l}")
                nc.sync.dma_start(wc_t[:], wc_d[l - 1])
                func = RELU
            new_act = [actpool.tile([128, G * G], F16, tag=f"act{b}",
                                    name=f"act{b}_{l}")
                       for b in range(BC)]
            if l < n_hid:
                gn = gpool.tile([128, BC, 16, T47], F16, tag="g", name=f"g_{l}")
                pn = ppool.tile([128, BC, T47], F16, tag="p", name=f"p_{l}")
                # rho=1 planes have only 15 data rows; zero the 16th
                nc.vector.memset(gn[64:128, :, 15:16, :], 0.0)
            else:
                gn = pn = None
            act_bs = new_act
            for b in range(BC):
                accs = [accp.tile([128, 512], F32, tag="acc",
                                  name=f"acc{l}_{b}{yh}") for yh in range(2)]
                if l == 0:
                    taps_l0(accs, q0x, b)
                else:
                    taps_tg(accs, g_cur, p_cur, w_t, wc_t, b)
                # deprioritize the epilogue chain so the next sample's (or
                # layer's) taps slide ahead of it in the engine queues; the
                # copy->reduce->act->build chain then hides under PE work
                with tc.high_priority(offset=-200):
                    for yh in range(2):
                        chunk_epilogue(l, b, yh, accs[yh], func)
                    if gn is not None:
                        build_pg(l, b, pn, gn)
            g_cur, p_cur = gn, pn

        # ---- 1x1 out conv
        for b in range(BC):
            for yh in range(2):
                po = p2p.tile([128, 512], F32, tag="p2", name=f"po{b}_{yh}")
                nc.tensor.matmul(
                    po[0:1, :], wo_t[:, :],
                    act_bs[b][0:32, yh * 512:(yh + 1) * 512],
                    start=True, stop=True)
                nc.scalar.activation(
                    osb[0:1, (2 * b + yh) * 512:(2 * b + yh + 1) * 512],
                    po[0:1, :], IDENT, bias=bia_t[0:1, n_hid + 1:n_hid + 2],
                    scale=float(GAIN ** -(n_hid + 1)))
        nc.sync.dma_start(out_d.rearrange("b one h w -> one (b h w)"), osb[:])

    return nc


_NC_CACHE = {}


def _get_nc(n_hid=N_HID):
    if n_hid not in _NC_CACHE:
        nc = _build_nc(n_hid)
        nc.finalize()
        _NC_CACHE[n_hid] = nc
    return _NC_CACHE[n_hid]


def _make_inmaps(x, in_w, in_b, convs_w, convs_b, out_w, out_b):
    n_hid = convs_w.shape[0]
    w0, wh, wc = _prep_weights(np.asarray(in_w, np.float32),
                               np.asarray(convs_w, np.float32))
    # fold a GAIN-per-layer renorm into the weights (undone by the out-conv
    # ACT scale) so fp16 activations stay O(1) through the 21-layer chain
    w0 = (w0 * GAIN).astype(np.float16)
    wh = (wh * GAIN).astype(np.float16)
    wc = (wc * GAIN).astype(np.float16)
    bia = np.zeros((n_hid + 2, 128), np.float32)
    bia[0] = np.tile(np.asarray(in_b, np.float32), 4) * GAIN
    for l in range(n_hid):
        bia[l + 1] = np.tile(np.asarray(convs_b[l], np.float32), 4) * GAIN ** (l + 2)
    bia[n_hid + 1] = np.asarray(out_b, np.float32)[0]
    smat = np.tile(np.eye(32, dtype=np.float16), (4, 4))
    wo = np.ascontiguousarray(
        np.asarray(out_w, np.float32)[0, :, 0, 0][:, None]).astype(np.float16)
    shared = {"w0": w0, "wh": wh, "wc": wc, "bia": bia, "smat": smat, "wo": wo}
    x = np.asarray(x, np.float32)
    return [dict(shared, q0x=_prep_q0x(x[i * BC:(i + 1) * BC]).astype(np.float16))
            for i in range(N_CORES)]


def _run_traced(nc, in_maps):
    """Execute via PJRT with NRT profiling (ctypes into the axon .so) and
    extract core-0 exec time from the NTFF via neuron-profile."""
    import glob
    import subprocess
    import tempfile

    from concourse import bass2jax

    sys.path.insert(0, "/root/.axon_site")
    from trn_agent_boot.trn_boot import _ntff_profile_via_ctypes

    hook = _ntff_profile_via_ctypes("/opt/axon/libaxon_pjrt.so")
    outdir = tempfile.mkdtemp(prefix="ntff_")
    with hook(outdir, [0]):
        results = bass2jax.run_bass_via_pjrt(nc, in_maps, n_cores=len(in_maps))
    exec_ns = None
    ntffs = sorted(glob.glob(os.path.join(outdir, "*.ntff")))
    neffs = glob.glob(os.path.join(outdir, "*.neff")) or [
        p for p in glob.glob(os.path.expanduser(
            "~/.neuron-compile-cache/**/model.neff"), recursive=True)
        if os.path.getsize(p) > (1 << 20)]
    if ntffs and neffs:
        neff = max(neffs, key=os.path.getmtime)
        out_json = os.path.join(outdir, "ntff.json")
        try:
            subprocess.run(
                ["neuron-profile", "view", "-n", neff, "-s", ntffs[0],
                 "--output-format=json", "--output-file", out_json,
                 "--ignore-nc-buf-usage"],
                check=True, capture_output=True,
                env=dict(os.environ, NEURON_PROFILE_DBG_OUTPUT="2"))
            exec_ns = _exec_ns_from_json(out_json)
        except Exception as e:  # profiling must never break the run
            print("profile extraction failed:", e)
    print("ntff dir:", outdir)
    return results, exec_ns


def _exec_ns_from_json(path):
    import json
    with open(path) as f:
        d = json.load(f)
    lo, hi = None, None
    insts = d.get("instruction") or d.get("instructions") or {}
    if isinstance(insts, dict):
        it = insts.values()
    else:
        it = insts
    for rec in it:
        try:
            t0 = int(rec["timestamp"])
            dur = int(rec.get("duration", 0))
        except (KeyError, TypeError, ValueError):
            continue
        lo = t0 if lo is None else min(lo, t0)
        hi = t0 + dur if hi is None else max(hi, t0 + dur)
    if lo is None:
        return None
    return hi - lo


def _run(inputs, trace=False, n_hid=N_HID):
    nc = _get_nc(n_hid)
    in_maps = _make_inmaps(**inputs)
    if trace:
        results, exec_ns = _run_traced(nc, in_maps)
    else:
        res = run_bass_kernel_spmd(nc, in_maps, list(range(N_CORES)),
                                   trace=False)
        results, exec_ns = res.results, res.exec_time_ns
    out = np.concatenate([results[i]["out"] for i in range(N_CORES)], axis=0)
    return np.asarray(out, np.float32), exec_ns


def kernel(**inputs):
    out, _ = _run(inputs)
    return out


# revision 52
# speedup vs baseline: 1.0056x; 1.0056x over previous
"""TRN2 Bass kernel for nn_Baseline_v4_74947179315922 (dense CNN, 22 conv layers).

Network: in_conv(k=32,s=2 on 31x-iterated-wrap-pad) -> 20 hidden convs
(k=32,s=2, ReLU) -> 1x1 out conv.  B=16, grid 32x32, 32 channels.

Strategy (per core, pure data parallelism, B_core=2):
 - The iterated wrap-pad (94x94) is parity-split into four 47x47 planes
   Q[rho,gamma][t,u] = h_pad[2t+rho, 2u+gamma]; the stride-2 32x32-kernel conv
   becomes 256 stride-1 "taps" (a,c) in [0,16)^2 contracting K=128 =
   (4 parities x 32 channels).
 - Border folding: Q[t,u] = P[u] + G[t,u] where P is the (constant) padded
   border row and G is supported on the 16 data rows only.  Each tap then
   streams just the <=16 valid output rows from G (half the dense stream),
   and the border term collapses to 16 "column taps" with row-summed weights
   and a y-broadcast (stride-0) rhs read of P.
 - Each pass runs 4 taps concurrently on the PE array via column tiling
   (tile_position=(0,32j)); PSUM accumulates across all taps; a
   [tile(eye(32),(4,4))] matmul reduces the 4 tap-groups.  The two y-half
   chunks of a tap share one weight slice and alternate PSUM banks, which
   pipelines weight loads and drains (measured ~14ns/MM overhead vs the
   213ns N=512 fp16 stream floor).
 - ReLU+bias applied by ScalarE straight out of PSUM; the next layer's P and
   G are built by a few DVE subtract/copy ops.  The epilogue chain is
   emitted at reduced scheduler priority (tc.high_priority(offset=-150)) so
   the next sample's tap stream slides ahead of it in the engine queues and
   hides the copy->reduce->act->build latency.
"""

import os
import sys

sys.path.insert(0, "/opt/trn_rl_repo")

from contextlib import ExitStack

import numpy as np

import concourse.bass as bass
import concourse.bacc as bacc
import concourse.tile as tile
from concourse import mybir
from concourse.bass_utils import run_bass_kernel_spmd

F32 = mybir.dt.float32
F16 = mybir.dt.float16
RELU = mybir.ActivationFunctionType.Relu
IDENT = mybir.ActivationFunctionType.Identity

N_CORES = 8
B = 16
BC = B // N_CORES          # samples per core
G = 32                     # grid
T47 = 47                   # parity-plane extent
N_HID = 20
GAIN = float(np.sqrt(2.0))   # per-layer activation renorm (ReLU halves variance)

# parity-plane row/col maps: plane rho=0 rows are [31]*16, 1,3..31, [31]*15;
# plane rho=1 rows are [0]*16, 2,4..30, [0]*16  (same for columns)
PMAP = {
    0: [31] * 16 + list(range(1, 32, 2)) + [31] * 15,
    1: [0] * 16 + list(range(2, 31, 2)) + [0] * 16,
}
# bands: (dst_start, dst_len, src_start, src_step)
BANDS = {
    0: [(0, 16, 31, 0), (16, 16, 1, 2), (32, 15, 31, 0)],
    1: [(0, 16, 0, 0), (16, 15, 2, 2), (31, 16, 0, 0)],
}


def _prep_weights(in_w, convs_w):
    """Host-side rearrangement into PE lhsT layouts."""
    n_hid = convs_w.shape[0]
    # hidden: [l, k=(rho*2+gamma)*32+i, p*128 + j*32 + o], tap = 4p+j = a*16+c
    w6 = convs_w.reshape(n_hid, 32, 32, 16, 2, 16, 2)      # l,o,i,a,rho,c,gamma
    wh6 = w6.transpose(0, 4, 6, 2, 3, 5, 1)                # l,rho,gamma,i,a,c,o
    wh = np.ascontiguousarray(wh6.reshape(n_hid, 128, 64 * 128), np.float32)
    # column taps: wc[l, k, c*32+o] = sum_a W  (border-row folding)
    wc = np.ascontiguousarray(
        wh6.sum(axis=4).reshape(n_hid, 128, 16 * 32), np.float32)
    # layer0: [k=(delta*4+rho*2+gamma), p*128 + j*32 + o], a=a0+delta, c=c0+j,
    # a0=(p//4)*8, c0=(p%4)*4
    u = in_w[:, 0].reshape(32, 2, 8, 2, 4, 4, 2)           # o,a0i,delta,rho,c0i,j,gamma
    w0 = u.transpose(2, 3, 6, 1, 4, 5, 0)                  # delta,rho,gamma,a0i,c0i,j,o
    w0 = np.ascontiguousarray(w0.reshape(32, 8 * 128), np.float32)
    return w0, wh, wc


def _prep_q0x(x):
    """Host-built layer-0 input: delta-shifted parity planes of padded x."""
    q0x = np.zeros((32, BC, T47, T47), np.float32)
    for g in range(4):
        rho, gam = g // 2, g % 2
        plane = x[:, 0][:, PMAP[rho]][:, :, PMAP[gam]]     # [BC,47,47]
        for d in range(8):
            q0x[4 * d + g, :, :T47 - d, :] = plane[:, d:, :]
    return q0x


def _build_nc(n_hid=N_HID):
    nc = bacc.Bacc("TRN2", target_bir_lowering=False)

    q0_d = nc.dram_tensor("q0x", [32, BC, T47, T47], F16, kind="ExternalInput")
    w0_d = nc.dram_tensor("w0", [32, 8 * 128], F16, kind="ExternalInput")
    wh_d = nc.dram_tensor("wh", [n_hid, 128, 64 * 128], F16, kind="ExternalInput")
    wc_d = nc.dram_tensor("wc", [n_hid, 128, 16 * 32], F16, kind="ExternalInput")
    # bias rows: 0 = in_b tiled x4; 1..n_hid = convs_b tiled x4; n_hid+1 = out_b
    bia_d = nc.dram_tensor("bia", [n_hid + 2, 128], F32, kind="ExternalInput")
    s_d = nc.dram_tensor("smat", [128, 128], F16, kind="ExternalInput")
    wo_d = nc.dram_tensor("wo", [32, 1], F16, kind="ExternalInput")
    out_d = nc.dram_tensor("out", [BC, 1, G, G], F32, kind="ExternalOutput")

    with tile.TileContext(nc) as tc, ExitStack() as ctx:
        const = ctx.enter_context(tc.tile_pool(name="const", bufs=1))
        gpool = ctx.enter_context(tc.tile_pool(name="gpool", bufs=2))
        ppool = ctx.enter_context(tc.tile_pool(name="ppool", bufs=2))
        wpool = ctx.enter_context(tc.tile_pool(name="wpool", bufs=2))
        wcpool = ctx.enter_context(tc.tile_pool(name="wcpool", bufs=2))
        s1pool = ctx.enter_context(tc.tile_pool(name="s1pool", bufs=2))
        actpool = ctx.enter_context(tc.tile_pool(name="actpool", bufs=2))
        accp = ctx.enter_context(tc.tile_pool(name="accp", bufs=4, space="PSUM"))
        p2p = ctx.enter_context(tc.tile_pool(name="p2p", bufs=3, space="PSUM"))

        q0x = const.tile([32, BC, T47, T47], F16)
        nc.sync.dma_start(q0x[:], q0_d[:])
        w0_t = const.tile([32, 8 * 128], F16)
        nc.sync.dma_start(w0_t[:], w0_d[:])
        s_t = const.tile([128, 128], F16)
        nc.sync.dma_start(s_t[:], s_d[:])
        bia_t = const.tile([128, n_hid + 2], F32)
        nc.sync.dma_start(bia_t[:], bia_d.rearrange("l p -> p l"))
        wo_t = const.tile([32, 1], F16)
        nc.sync.dma_start(wo_t[:], wo_d[:])
        osb = const.tile([1, BC * G * G], F32)

        def chunk_epilogue(l, b, yh, acc, func):
            """reduce 4 tap groups of one chunk, bias+act into act_b."""
            s1 = s1pool.tile([128, 512], F16, tag="s1")
            nc.vector.tensor_copy(s1[:], acc[:])
            p2 = p2p.tile([128, 512], F32, tag="p2")
            nc.tensor.matmul(p2[:], s_t[:, :],
                             s1[:, :], start=True, stop=True)
            nc.scalar.activation(
                act_bs[b][:, yh * 512:(yh + 1) * 512],
                p2[:], func, bias=bia_t[:, l:l + 1])

        def build_pg(l, b, pn, gn):
            """build next layer's P (border row) and G (centered data rows)."""
            act3 = act_bs[b].rearrange("p (r c) -> p r c", r=G)
            # P: padded border row per plane (row 31 for rho=0, row 0 for rho=1)
            for g in range(4):
                rho, gam = g // 2, g % 2
                R = 31 if rho == 0 else 0
                for u0, ul, us, ust in BANDS[gam]:
                    if ust == 0:
                        src = act3[32 * g:32 * (g + 1), R, us:us + 1]
                        src = src.broadcast_to([32, ul])
                    else:
                        src = act3[32 * g:32 * (g + 1), R, us:us + 2 * ul - 1:2]
                    nc.vector.tensor_copy(
                        pn[32 * g:32 * (g + 1), b, u0:u0 + ul], src)
            # G bands: center = data - P; borders = rowborder - corner
            for g in range(4):
                rho, gam = g // 2, g % 2
                nr = 16 if rho == 0 else 15
                rs = 1 if rho == 0 else 2
                C = 31 if gam == 0 else 0
                rows = act3[32 * g:32 * (g + 1), rs:rs + 2 * nr - 1:2, :]
                u0c, wc_, usc, _ = BANDS[gam][1]
                nc.vector.tensor_sub(
                    gn[32 * g:32 * (g + 1), b, 0:nr, u0c:u0c + wc_],
                    rows[:, :, usc:usc + 2 * wc_ - 1:2],
                    pn[32 * g:32 * (g + 1), b, u0c:u0c + wc_]
                    .unsqueeze(1).broadcast_to([32, nr, wc_]))
                for u0, ul, us, ust in (BANDS[gam][0], BANDS[gam][2]):
                    nc.vector.tensor_sub(
                        gn[32 * g:32 * (g + 1), b, 0:nr, u0:u0 + ul],
                        rows[:, :, C:C + 1].broadcast_to([32, nr, ul]),
                        pn[32 * g:32 * (g + 1), b, 0:1]
                        .unsqueeze(1).broadcast_to([32, nr, ul]))

        def taps_l0(accs2, q_ap, b):
            """dense layer-0 taps (K=32, delta folded into K, 8 passes)."""
            for p in range(8):
                for j in range(4):
                    a = (p // 4) * 8
                    c = (p % 4) * 4 + j
                    w_ap = w0_t[:, p * 128 + 32 * j:p * 128 + 32 * (j + 1)]
                    for yh in range(2):
                        rhs = q_ap[0:32, b, yh * 16 + a:yh * 16 + a + 16,
                                   c:c + 32]
                        nc.tensor.matmul(
                            accs2[yh][32 * j:32 * (j + 1), :],
                            w_ap, rhs,
                            start=(p == 0), stop=(p == 7),
                            tile_position=(0, 32 * j), skip_group_check=True)

        def taps_tg(accs2, g_t, p_t, w_t, wc_t, b):
            """hidden-layer taps on G (16 data rows) + P column taps.

            Per-tap bank alternation (yh1 then yh0) pipelines PSUM drains.
            P column taps run FIRST: they only need P (built before G in the
            previous layer's build_pg), so the PE starts this sample's stream
            before its G tile finishes building at layer boundaries.
            """
            # P column taps: y-broadcast rhs, row-summed weights
            for pc in range(4):
                for j in range(4):
                    c = pc * 4 + j
                    wc_ap = wc_t[:, c * 32:(c + 1) * 32]
                    rhs = (p_t[0:128, b, c:c + 32]
                           .unsqueeze(1).broadcast_to([128, 16, 32]))
                    for yh in range(2):
                        nc.tensor.matmul(
                            accs2[yh][32 * j:32 * (j + 1), :],
                            wc_ap, rhs,
                            start=(pc == 0), stop=False,
                            tile_position=(0, 32 * j), skip_group_check=True)
            for p in range(64):
                for j in range(4):
                    tau = 4 * p + j
                    a, c = tau // 16, tau % 16
                    w_ap = w_t[:, p * 128 + 32 * j:p * 128 + 32 * (j + 1)]
                    # chunk yh1: out rows [16,32-a), G rows [a,16)
                    nc.tensor.matmul(
                        accs2[1][32 * j:32 * (j + 1), 0:(16 - a) * 32],
                        w_ap, g_t[0:128, b, a:16, c:c + 32],
                        start=False, stop=(p == 63),
                        tile_position=(0, 32 * j), skip_group_check=True)
                    # chunk yh0: out rows [16-a,16), G rows [0,a)
                    if a >= 1:
                        nc.tensor.matmul(
                            accs2[0][32 * j:32 * (j + 1), (16 - a) * 32:512],
                            w_ap, g_t[0:128, b, 0:a, c:c + 32],
                            start=False, stop=(p == 63),
                            tile_position=(0, 32 * j), skip_group_check=True)

        # ---- all conv layers (layer 0 + hidden); per-b epilogue interleave
        g_cur = p_cur = None
        act_bs = None
        for l in range(n_hid + 1):
            if l == 0:
                w_t = wc_t = None
                func = IDENT
            else:
                w_t = wpool.tile([128, 64 * 128], F16, tag="wh", name=f"wh{l}")
                nc.sync.dma_start(w_t[:], wh_d[l - 1])
                wc_t = wcpool.tile([128, 16 * 32], F16, tag="wc", name=f"wc{l}")
                nc.sync.dma_start(wc_t[:], wc_d[l - 1])
                func = RELU
            new_act = [actpool.tile([128, G * G], F16, tag=f"act{b}",
                                    name=f"act{b}_{l}")
                       for b in range(BC)]
            if l < n_hid:
                gn = gpool.tile([128, BC, 16, T47], F16, tag="g", name=f"g_{l}")
                pn = ppool.tile([128, BC, T47], F16, tag="p", name=f"p_{l}")
                # rho=1 planes have only 15 data rows; zero the 16th
                nc.vector.memset(gn[64:128, :, 15:16, :], 0.0)
            else:
                gn = pn = None
            act_bs = new_act
            for b in range(BC):
                accs = [accp.tile([128, 512], F32, tag="acc",
                                  name=f"acc{l}_{b}{yh}") for yh in range(2)]
                if l == 0:
                    taps_l0(accs, q0x, b)
                else:
                    taps_tg(accs, g_cur, p_cur, w_t, wc_t, b)
                # deprioritize the epilogue chain so the next sample's (or
                # layer's) taps slide ahead of it in the engine queues; the
                # copy->reduce->act->build chain then hides under PE work
                with tc.high_priority(offset=-150):
                    for yh in range(2):
                        chunk_epilogue(l, b, yh, accs[yh], func)
                    if gn is not None:
                        build_pg(l, b, pn, gn)
            g_cur, p_cur = gn, pn

        # ---- 1x1 out conv
        for b in range(BC):
            for yh in range(2):
                po = p2p.tile([128, 512], F32, tag="p2", name=f"po{b}_{yh}")
                nc.tensor.matmul(
                    po[0:1, :], wo_t[:, :],
                    act_bs[b][0:32, yh * 512:(yh + 1) * 512],
                    start=True, stop=True)
                nc.scalar.activation(
                    osb[0:1, (2 * b + yh) * 512:(2 * b + yh + 1) * 512],
                    po[0:1, :], IDENT, bias=bia_t[0:1, n_hid + 1:n_hid + 2],
                    scale=float(GAIN ** -(n_hid + 1)))
        nc.sync.dma_start(out_d.rearrange("b one h w -> one (b h w)"), osb[:])

    return nc


_NC_CACHE = {}


def _get_nc(n_hid=N_HID):
    if n_hid not in _NC_CACHE:
        nc = _build_nc(n_hid)
        nc.finalize()
        _NC_CACHE[n_hid] = nc
    return _NC_CACHE[n_hid]


def _make_inmaps(x, in_w, in_b, convs_w, convs_b, out_w, out_b):
    n_hid = convs_w.shape[0]
    w0, wh, wc = _prep_weights(np.asarray(in_w, np.float32),
                               np.asarray(convs_w, np.float32))
    # fold a GAIN-per-layer renorm into the weights (undone by the out-conv
    # ACT scale) so fp16 activations stay O(1) through the 21-layer chain
    w0 = (w0 * GAIN).astype(np.float16)
    wh = (wh * GAIN).astype(np.float16)
    wc = (wc * GAIN).astype(np.float16)
    bia = np.zeros((n_hid + 2, 128), np.float32)
    bia[0] = np.tile(np.asarray(in_b, np.float32), 4) * GAIN
    for l in range(n_hid):
        bia[l + 1] = np.tile(np.asarray(convs_b[l], np.float32), 4) * GAIN ** (l + 2)
    bia[n_hid + 1] = np.asarray(out_b, np.float32)[0]
    smat = np.tile(np.eye(32, dtype=np.float16), (4, 4))
    wo = np.ascontiguousarray(
        np.asarray(out_w, np.float32)[0, :, 0, 0][:, None]).astype(np.float16)
    shared = {"w0": w0, "wh": wh, "wc": wc, "bia": bia, "smat": smat, "wo": wo}
    x = np.asarray(x, np.float32)
    return [dict(shared, q0x=_prep_q0x(x[i * BC:(i + 1) * BC]).astype(np.float16))
            for i in range(N_CORES)]


def _run_traced(nc, in_maps):
    """Execute via PJRT with NRT profiling (ctypes into the axon .so) and
    extract core-0 exec time from the NTFF via neuron-profile."""
    import glob
    import subprocess
    import tempfile

    from concourse import bass2jax

    sys.path.insert(0, "/root/.axon_site")
    from trn_agent_boot.trn_boot import _ntff_profile_via_ctypes

    hook = _ntff_profile_via_ctypes("/opt/axon/libaxon_pjrt.so")
    outdir = tempfile.mkdtemp(prefix="ntff_")
    with hook(outdir, [0]):
        results = bass2jax.run_bass_via_pjrt(nc, in_maps, n_cores=len(in_maps))
    exec_ns = None
    ntffs = sorted(glob.glob(os.path.join(outdir, "*.ntff")))
    neffs = glob.glob(os.path.join(outdir, "*.neff")) or [
        p for p in glob.glob(os.path.expanduser(
            "~/.neuron-compile-cache/**/model.neff"), recursive=True)
        if os.path.getsize(p) > (1 << 20)]
    if ntffs and neffs:
        neff = max(neffs, key=os.path.getmtime)
        out_json = os.path.join(outdir, "ntff.json")
        try:
            subprocess.run(
                ["neuron-profile", "view", "-n", neff, "-s", ntffs[0],
                 "--output-format=json", "--output-file", out_json,
                 "--ignore-nc-buf-usage"],
                check=True, capture_output=True,
                env=dict(os.environ, NEURON_PROFILE_DBG_OUTPUT="2"))
            exec_ns = _exec_ns_from_json(out_json)
        except Exception as e:  # profiling must never break the run
            print("profile extraction failed:", e)
    print("ntff dir:", outdir)
    return results, exec_ns


def _exec_ns_from_json(path):
    import json
    with open(path) as f:
        d = json.load(f)
    lo, hi = None, None
    insts = d.get("instruction") or d.get("instructions") or {}
    if isinstance(insts, dict):
        it = insts.values()
    else:
        it = insts
    for rec in it:
        try:
            t0 = int(rec["timestamp"])
            dur = int(rec.get("duration", 0))
        except (KeyError, TypeError, ValueError):
            continue
        lo = t0 if lo is None else min(lo, t0)
        hi = t0 + dur if hi is None else max(hi, t0 + dur)
    if lo is None:
        return None
    return hi - lo


def _run(inputs, trace=False, n_hid=N_HID):
    nc = _get_nc(n_hid)
    in_maps = _make_inmaps(**inputs)
    if trace:
        results, exec_ns = _run_traced(nc, in_maps)
    else:
        res = run_bass_kernel_spmd(nc, in_maps, list(range(N_CORES)),
                                   trace=False)
        results, exec_ns = res.results, res.exec_time_ns
    out = np.concatenate([results[i]["out"] for i in range(N_CORES)], axis=0)
    return np.asarray(out, np.float32), exec_ns


def kernel(**inputs):
    out, _ = _run(inputs)
    return out
